# revision 11
# baseline (speedup 1.0000x reference)
"""Depthwise 4x4 blur (upfirdn2d pad=(2,1)) on TRN2, 8 NeuronCores.

The 2D blur kernel [1,3,3,1]x[1,3,3,1]/64 is separable, so
out = Av @ X @ Ah^T per image, where Av/Ah are 128x128 banded matrices
(4-tap band; H/W zero-padding folded into the band clipping). On the PE:

  pass 1:  tmpT = lhsT.T @ rhs with lhsT = X (the image as the STATIONARY
           operand), rhs = Av^T          -> tmpT = (Av @ X)^T   [w, h] PSUM
  pass 2:  outT = lhsT.T @ rhs with lhsT = Ah^T, rhs = tmpT (4 images)
                                         -> outT = (Av@X@Ah^T)^T [w, h] PSUM

Putting the per-image matrix on the stationary side in pass 1 means NO
transposes anywhere: the output simply leaves the device W-major and the
host untransposes for free. PE streams 256 cols/image (vs 1024 for the
4-banded-matmul hilo formulation) plus one 128-col LDWEIGHTS per image
(FWL, overlapped with the previous matmul via the background weight
buffer).

Everything on-chip is fp16 (PSUM accumulation stays fp32): rel err ~7e-4
vs the 2e-2 gate, and HBM traffic halves vs f32 (16.8 MB/core total).
Host pre-arranges x as [H, C, W] fp16 so every DMA row is a contiguous
4 KB per partition (the f32 baseline's 524 B rows capped each HWDGE ring
at ~190 GB/s). Input DMAs ride the SP HWDGE ring, output DMAs the
GpSimd SWDGE path, leaving ACT/DVE free for PSUM evacuation.

Sharding: batch dim (8 batches -> 8 cores), 256 images of 128x128 each.
"""

import numpy as np
from contextlib import ExitStack

import concourse.bass as bass
import concourse.bacc as bacc
import concourse.tile as tile
import concourse.mybir as mybir
from concourse.bass_utils import run_bass_kernel_spmd

N_CORES = 8
B, C, H, W = 8, 256, 128, 128
WP = W + 3         # padded image stride: [0, 0, x0..x127, 0]
GROUP = 4          # images per pass-2 matmul / PSUM bank (4*128 = 512 f32)
PAIR = 8           # images per pass-2 PSUM tile (2 banks) / ACT copy
SUPER = 16         # images per DMA (524 KB transfers)
N_DIRECT = 64      # trailing images on the direct 4-matmul path (PE-heavy,
                   # one PSUM evacuation) to offload the DVE/ACT copy engines
MODE = "sep16"

F32 = mybir.dt.float32
F16 = mybir.dt.float16


def _body_sep16(
    ctx, tc, os_ap, od_ap, x_ap, w_ap, n_direct=N_DIRECT, out_eng="gpsimd"
):
    nc = tc.nc
    S = C - n_direct   # images [0,S) separable path, [S,C) direct path
    wpool = ctx.enter_context(tc.tile_pool(name="wts", bufs=1))
    # deep input prefetch: the whole fp16 input fits in SBUF, so let the
    # input ring run back-to-back instead of throttling on compute
    xpool = ctx.enter_context(tc.tile_pool(name="xin", bufs=16))
    tpool = ctx.enter_context(tc.tile_pool(name="tmid", bufs=8))
    opool = ctx.enter_context(tc.tile_pool(name="oup", bufs=4))
    p1pool = ctx.enter_context(tc.tile_pool(name="ps1", bufs=4, space="PSUM"))
    p2pool = ctx.enter_context(tc.tile_pool(name="ps2", bufs=2, space="PSUM"))

    wt = wpool.tile([H, 6 * H], F16)
    nc.scalar.dma_start(wt[:], w_ap)
    wv = wt[:, :H]         # Av^T: moving operand of sep pass 1
    wh = wt[:, H : 2 * H]  # Ah^T: stationary operand of sep pass 2
    wd = [wt[:, (2 + j) * H : (3 + j) * H] for j in range(4)]  # direct lhsT_j

    oeng = {"gpsimd": nc.gpsimd, "scalar": nc.scalar, "sync": nc.sync}[out_eng]

    sizes_sep = [8, 8] + [SUPER] * ((S - 16) // SUPER)
    assert sum(sizes_sep) == S
    sizes_dir = [SUPER] * ((n_direct - 16) // SUPER) + [8, 8]
    assert sum(sizes_dir) == n_direct

    c0 = 0
    for sz in sizes_sep:
        xt = xpool.tile([H, sz * WP], F16, tag="xt")
        nc.sync.dma_start(
            xt[:].rearrange("h (c w) -> h c w", c=sz), x_ap[:, c0 : c0 + sz]
        )
        ot = opool.tile([H, sz * H], F16, tag="ot")
        for p0 in range(0, sz, PAIR):
            pc = min(PAIR, sz - p0)
            # pass 1: per-image stationary; 1-bank PSUM groups, DVE copies
            # (DVE 2-bank copies are slower than 2x 1-bank; ACT is opposite)
            tts = []
            for g in range(p0, p0 + pc, GROUP):
                gc = min(GROUP, p0 + pc - g)
                pt1 = p1pool.tile([H, gc * H], F32, tag="pt1")
                for i in range(gc):
                    c = g + i
                    nc.tensor.matmul(
                        pt1[:, i * H : (i + 1) * H],
                        xt[:, c * WP + 2 : c * WP + 2 + W],
                        wv,
                        start=True,
                        stop=True,
                    )
                tt = tpool.tile([H, gc * H], F16, tag="tt")
                nc.vector.tensor_copy(tt[:], pt1[:])
                tts.append((tt, gc))
            # pass 2: fixed stationary, 2-bank PSUM tile, one ACT copy
            pt2 = p2pool.tile([H, pc * H], F32, tag="pt2")
            o = 0
            for tt, gc in tts:
                nc.tensor.matmul(
                    pt2[:, o * H : (o + gc) * H],
                    wh,
                    tt[:],
                    start=True,
                    stop=True,
                )
                o += gc
            nc.scalar.copy(ot[:, p0 * H : (p0 + pc) * H], pt2[:])
        oeng.dma_start(
            os_ap[:, c0 : c0 + sz], ot[:].rearrange("w (c h) -> w c h", c=sz)
        )
        c0 += sz

    # direct path: OUT = sum_j A_j @ Xpad[:, j:j+W], natural [h, w] output,
    # one PSUM evacuation per 4 images, alternating DVE/ACT
    k = 0
    for sz in sizes_dir:
        xt = xpool.tile([H, sz * WP], F16, tag="xt")
        nc.sync.dma_start(
            xt[:].rearrange("h (c w) -> h c w", c=sz), x_ap[:, c0 : c0 + sz]
        )
        xt3 = xt[:].rearrange("h (c w) -> h c w", c=sz)
        ot = opool.tile([H, sz * W], F16, tag="ot")
        for g0 in range(0, sz, GROUP):
            pt = p1pool.tile([H, GROUP * W], F32, tag="pt1")
            for j in range(4):
                nc.tensor.matmul(
                    pt[:],
                    wd[j],
                    xt3[:, g0 : g0 + GROUP, j : j + W],
                    start=(j == 0),
                    stop=(j == 3),
                )
            dst = ot[:, g0 * W : (g0 + GROUP) * W]
            if k % 2 == 0:
                nc.vector.tensor_copy(dst, pt[:])
            else:
                nc.scalar.copy(dst, pt[:])
            k += 1
        oeng.dma_start(
            od_ap[:, c0 - S : c0 - S + sz],
            ot[:].rearrange("h (c w) -> h c w", c=sz),
        )
        c0 += sz


def build_module(mode=MODE, n_direct=N_DIRECT, **kw):
    nc = bacc.Bacc(
        "TRN2", target_bir_lowering=False, debug=False, num_devices=N_CORES
    )
    x_ap = nc.dram_tensor("x", [H, C, WP], F16, kind="ExternalInput").ap()
    w_ap = nc.dram_tensor("wts", [H, 6 * H], F16, kind="ExternalInput").ap()
    os_ap = nc.dram_tensor(
        "out_sep", [W, C - n_direct, H], F16, kind="ExternalOutput"
    ).ap()
    od_ap = nc.dram_tensor(
        "out_dir", [H, n_direct, W], F16, kind="ExternalOutput"
    ).ap()
    with tile.TileContext(nc) as tc:
        with ExitStack() as ctx:
            _body_sep16(ctx, tc, os_ap, od_ap, x_ap, w_ap, n_direct=n_direct, **kw)
    nc.compile()
    return nc


def band_mat(taps):
    """A[h, h+i-2] = taps[::-1][i], rows/cols clipped to [0,128)."""
    kf = np.asarray(taps, np.float32)[::-1]
    A = np.zeros((H, H), np.float32)
    for i in range(len(kf)):
        d = i - 2
        h0, h1 = max(0, -d), min(H, H - d)
        idx = np.arange(h0, h1)
        A[idx, idx + d] = kf[i]
    return A


def band_mats_2d(k2d):
    """Direct-path stationaries: WT[j] = A_j^T, A_j[h, h+i-2] = kf2d[i, j]."""
    kf = np.asarray(k2d, np.float32)[::-1, ::-1]
    wts = np.zeros((4, H, H), np.float32)
    for j in range(4):
        for i in range(4):
            d = i - 2
            h0, h1 = max(0, -d), min(H, H - d)
            idx = np.arange(h0, h1)
            wts[j, idx + d, idx] = kf[i, j]
    return wts


_module_cache = {}


def _get_module(mode=MODE, **kw):
    key = (mode, tuple(sorted(kw.items())))
    if key not in _module_cache:
        _module_cache[key] = build_module(mode, **kw)
    return _module_cache[key]


def kernel(x, kernel, _trace=False, _trace_kwargs=None, _mode=None, _build_kw=None):
    x = np.asarray(x)
    assert x.shape == (B, C, H, W), x.shape
    k2d = np.asarray(kernel, np.float32)
    # rank-1 factorization of the (sum-normalized) separable 2D kernel
    av = k2d.sum(1)
    ah = k2d.sum(0) / k2d.sum()
    wts = np.concatenate(
        [band_mat(av).T, band_mat(ah).T] + list(band_mats_2d(k2d)), axis=1
    ).astype(np.float16)
    xT = np.zeros((B, H, C, WP), np.float16)
    xT[..., 2 : 2 + W] = x.transpose(0, 2, 1, 3)
    bkw = dict(_build_kw or {})
    n_direct = bkw.get("n_direct", N_DIRECT)
    S = C - n_direct
    nc = _get_module(_mode or MODE, **bkw)
    in_maps = [{"x": xT[i], "wts": wts} for i in range(N_CORES)]
    res = run_bass_kernel_spmd(
        nc, in_maps, list(range(N_CORES)), trace=_trace, **(_trace_kwargs or {})
    )
    out = np.empty((B, C, H, W), np.float32)
    for i in range(N_CORES):
        # out_sep [W, S, H] -> [S, H, W]; out_dir [H, D, W] -> [D, H, W]
        out[i, :S] = res.results[i]["out_sep"].transpose(1, 2, 0)
        out[i, S:] = res.results[i]["out_dir"].transpose(1, 0, 2)
    if _trace:
        return out, res
    return out


# revision 13
# speedup vs baseline: 1.1145x; 1.1145x over previous
"""Depthwise 4x4 blur (upfirdn2d pad=(2,1)) on TRN2, 8 NeuronCores.

The 2D blur kernel [1,3,3,1]x[1,3,3,1]/64 is separable, so
out = Av @ X @ Ah^T per image, where Av/Ah are 128x128 banded matrices
(4-tap band; H/W zero-padding folded into the band clipping). On the PE:

  pass 1:  tmpT = lhsT.T @ rhs with lhsT = X (the image as the STATIONARY
           operand), rhs = Av^T          -> tmpT = (Av @ X)^T   [w, h] PSUM
  pass 2:  outT = lhsT.T @ rhs with lhsT = Ah^T, rhs = tmpT (4 images)
                                         -> outT = (Av@X@Ah^T)^T [w, h] PSUM

Putting the per-image matrix on the stationary side in pass 1 means NO
transposes anywhere: the output simply leaves the device W-major and the
host untransposes for free. PE streams 256 cols/image (vs 1024 for the
4-banded-matmul hilo formulation) plus one 128-col LDWEIGHTS per image
(FWL, overlapped with the previous matmul via the background weight
buffer).

Everything on-chip is fp16 (PSUM accumulation stays fp32): rel err ~7e-4
vs the 2e-2 gate, and HBM traffic halves vs f32 (16.8 MB/core total).
Host pre-arranges x as [H, C, W] fp16 so every DMA row is a contiguous
4 KB per partition (the f32 baseline's 524 B rows capped each HWDGE ring
at ~190 GB/s). Input DMAs ride the SP HWDGE ring, output DMAs the
GpSimd SWDGE path, leaving ACT/DVE free for PSUM evacuation.

Sharding: batch dim (8 batches -> 8 cores), 256 images of 128x128 each.
"""

import numpy as np
from contextlib import ExitStack

import concourse.bass as bass
import concourse.bacc as bacc
import concourse.tile as tile
import concourse.mybir as mybir
from concourse.bass_utils import run_bass_kernel_spmd

N_CORES = 8
B, C, H, W = 8, 256, 128, 128
WP = W + 3         # padded image stride: [0, 0, x0..x127, 0]
GROUP = 4          # images per pass-2 matmul / PSUM bank (4*128 = 512 f32)
PAIR = 8           # images per pass-2 PSUM tile (2 banks) / ACT copy
SUPER = 16         # images per DMA (524 KB transfers)
N_DIRECT = 64      # trailing images on the direct 4-matmul path (PE-heavy,
                   # one PSUM evacuation) to offload the DVE/ACT copy engines
MODE = "sep16"

F32 = mybir.dt.float32
F16 = mybir.dt.float16


def _body_sep16(
    ctx, tc, os_ap, od_ap, x_ap, w_ap, n_direct=N_DIRECT, out_eng="gpsimd"
):
    nc = tc.nc
    S = C - n_direct   # images [0,S) separable path, [S,C) direct path
    wpool = ctx.enter_context(tc.tile_pool(name="wts", bufs=1))
    # deep input prefetch: the whole fp16 input fits in SBUF, so let the
    # input ring run back-to-back instead of throttling on compute
    xpool = ctx.enter_context(tc.tile_pool(name="xin", bufs=16))
    tpool = ctx.enter_context(tc.tile_pool(name="tmid", bufs=8))
    opool = ctx.enter_context(tc.tile_pool(name="oup", bufs=4))
    p1pool = ctx.enter_context(tc.tile_pool(name="ps1", bufs=4, space="PSUM"))
    p2pool = ctx.enter_context(tc.tile_pool(name="ps2", bufs=2, space="PSUM"))

    wt = wpool.tile([H, 6 * H], F16)
    nc.scalar.dma_start(wt[:], w_ap)
    wv = wt[:, :H]         # Av^T: moving operand of sep pass 1
    wh = wt[:, H : 2 * H]  # Ah^T: stationary operand of sep pass 2
    wd = [wt[:, (2 + j) * H : (3 + j) * H] for j in range(4)]  # direct lhsT_j

    oeng = {"gpsimd": nc.gpsimd, "scalar": nc.scalar, "sync": nc.sync}[out_eng]

    # each supertile mixes both paths so every engine is loaded uniformly:
    # sep images are copy-engine-heavy, direct images are PE-heavy
    plan = [8, 8] + [SUPER] * ((C - 32) // SUPER) + [8, 8]
    assert sum(plan) == C

    c0 = s0 = d0 = 0
    k = 0
    for sz in plan:
        nd = sz * n_direct // C
        ns = sz - nd
        xt = xpool.tile([H, sz * WP], F16, tag="xt")
        nc.sync.dma_start(
            xt[:].rearrange("h (c w) -> h c w", c=sz), x_ap[:, c0 : c0 + sz]
        )
        xt3 = xt[:].rearrange("h (c w) -> h c w", c=sz)

        # ---- separable path: images [c0, c0+ns) ----
        ots = opool.tile([H, ns * H], F16, tag="ots")
        for p0 in range(0, ns, PAIR):
            pc = min(PAIR, ns - p0)
            # pass 1: per-image stationary; 1-bank PSUM groups, DVE copies
            # (DVE 2-bank copies are slower than 2x 1-bank; ACT is opposite)
            tts = []
            for g in range(p0, p0 + pc, GROUP):
                gc = min(GROUP, p0 + pc - g)
                pt1 = p1pool.tile([H, gc * H], F32, tag="pt1")
                for i in range(gc):
                    c = g + i
                    nc.tensor.matmul(
                        pt1[:, i * H : (i + 1) * H],
                        xt[:, c * WP + 2 : c * WP + 2 + W],
                        wv,
                        start=True,
                        stop=True,
                    )
                tt = tpool.tile([H, gc * H], F16, tag="tt")
                nc.vector.tensor_copy(tt[:], pt1[:])
                tts.append((tt, gc))
            # pass 2: fixed stationary, 2-bank PSUM tile, one ACT copy
            pt2 = p2pool.tile([H, pc * H], F32, tag="pt2")
            o = 0
            for tt, gc in tts:
                nc.tensor.matmul(
                    pt2[:, o * H : (o + gc) * H],
                    wh,
                    tt[:],
                    start=True,
                    stop=True,
                )
                o += gc
            nc.scalar.copy(ots[:, p0 * H : (p0 + pc) * H], pt2[:])
        oeng.dma_start(
            os_ap[:, s0 : s0 + ns], ots[:].rearrange("w (c h) -> w c h", c=ns)
        )

        # ---- direct path: images [c0+ns, c0+sz), natural [h, w] output,
        # one PSUM evacuation per 4 images, alternating DVE/ACT ----
        if nd:
            otd = opool.tile([H, nd * W], F16, tag="otd")
            for g0 in range(ns, sz, GROUP):
                gc = min(GROUP, sz - g0)
                pt = p1pool.tile([H, gc * W], F32, tag="pt1")
                for j in range(4):
                    nc.tensor.matmul(
                        pt[:],
                        wd[j],
                        xt3[:, g0 : g0 + gc, j : j + W],
                        start=(j == 0),
                        stop=(j == 3),
                    )
                dst = otd[:, (g0 - ns) * W : (g0 - ns + gc) * W]
                if k % 2 == 0:
                    nc.vector.tensor_copy(dst, pt[:])
                else:
                    nc.scalar.copy(dst, pt[:])
                k += 1
            oeng.dma_start(
                od_ap[:, d0 : d0 + nd],
                otd[:].rearrange("h (c w) -> h c w", c=nd),
            )
        c0 += sz
        s0 += ns
        d0 += nd


def build_module(mode=MODE, n_direct=N_DIRECT, **kw):
    nc = bacc.Bacc(
        "TRN2", target_bir_lowering=False, debug=False, num_devices=N_CORES
    )
    x_ap = nc.dram_tensor("x", [H, C, WP], F16, kind="ExternalInput").ap()
    w_ap = nc.dram_tensor("wts", [H, 6 * H], F16, kind="ExternalInput").ap()
    os_ap = nc.dram_tensor(
        "out_sep", [W, C - n_direct, H], F16, kind="ExternalOutput"
    ).ap()
    od_ap = nc.dram_tensor(
        "out_dir", [H, n_direct, W], F16, kind="ExternalOutput"
    ).ap()
    with tile.TileContext(nc) as tc:
        with ExitStack() as ctx:
            _body_sep16(ctx, tc, os_ap, od_ap, x_ap, w_ap, n_direct=n_direct, **kw)
    nc.compile()
    return nc


def band_mat(taps):
    """A[h, h+i-2] = taps[::-1][i], rows/cols clipped to [0,128)."""
    kf = np.asarray(taps, np.float32)[::-1]
    A = np.zeros((H, H), np.float32)
    for i in range(len(kf)):
        d = i - 2
        h0, h1 = max(0, -d), min(H, H - d)
        idx = np.arange(h0, h1)
        A[idx, idx + d] = kf[i]
    return A


def band_mats_2d(k2d):
    """Direct-path stationaries: WT[j] = A_j^T, A_j[h, h+i-2] = kf2d[i, j]."""
    kf = np.asarray(k2d, np.float32)[::-1, ::-1]
    wts = np.zeros((4, H, H), np.float32)
    for j in range(4):
        for i in range(4):
            d = i - 2
            h0, h1 = max(0, -d), min(H, H - d)
            idx = np.arange(h0, h1)
            wts[j, idx + d, idx] = kf[i, j]
    return wts


_module_cache = {}


def _get_module(mode=MODE, **kw):
    key = (mode, tuple(sorted(kw.items())))
    if key not in _module_cache:
        _module_cache[key] = build_module(mode, **kw)
    return _module_cache[key]


def kernel(x, kernel, _trace=False, _trace_kwargs=None, _mode=None, _build_kw=None):
    x = np.asarray(x)
    assert x.shape == (B, C, H, W), x.shape
    k2d = np.asarray(kernel, np.float32)
    # rank-1 factorization of the (sum-normalized) separable 2D kernel
    av = k2d.sum(1)
    ah = k2d.sum(0) / k2d.sum()
    wts = np.concatenate(
        [band_mat(av).T, band_mat(ah).T] + list(band_mats_2d(k2d)), axis=1
    ).astype(np.float16)
    xT = np.zeros((B, H, C, WP), np.float16)
    xT[..., 2 : 2 + W] = x.transpose(0, 2, 1, 3)
    bkw = dict(_build_kw or {})
    n_direct = bkw.get("n_direct", N_DIRECT)
    S = C - n_direct
    nc = _get_module(_mode or MODE, **bkw)
    in_maps = [{"x": xT[i], "wts": wts} for i in range(N_CORES)]
    res = run_bass_kernel_spmd(
        nc, in_maps, list(range(N_CORES)), trace=_trace, **(_trace_kwargs or {})
    )
    # reconstruct the interleaved sep/direct channel assignment
    plan = [8, 8] + [SUPER] * ((C - 32) // SUPER) + [8, 8]
    sep_ch, dir_ch = [], []
    c0 = 0
    for sz in plan:
        nd = sz * n_direct // C
        sep_ch += range(c0, c0 + sz - nd)
        dir_ch += range(c0 + sz - nd, c0 + sz)
        c0 += sz
    out = np.empty((B, C, H, W), np.float32)
    for i in range(N_CORES):
        # out_sep [W, S, H] -> [S, H, W]; out_dir [H, D, W] -> [D, H, W]
        out[i, sep_ch] = res.results[i]["out_sep"].transpose(1, 2, 0)
        if dir_ch:
            out[i, dir_ch] = res.results[i]["out_dir"].transpose(1, 0, 2)
    if _trace:
        return out, res
    return out
